# revision 20
# baseline (speedup 1.0000x reference)
"""Gaussian self-attention Trainium2 kernel (8-core data-parallel over batch).

Module: scores[i,j,h,k,l] = u_h . [dx, dy, dx^2, dy^2, dx*dy], dx=k-i, dy=l-j
        probs = softmax over (k,l); vals = probs @ hidden; out = vals @ W^T + b

Key structure: scores depend only on (dx, dy) in [-31,31]^2, so the softmax
numerator is a 63x63 table per head and the denominator Z a 32x32 box-sum.
The host precomputes (from the tiny parameter tensors) the exp tables and 1/Z;
the device materializes nothing: each core DMA-loads a per-partition shifted
strip S[p, u] = tab[63*(p%4) + (p//4) + u] (one wide strip per head, covering
both ij-halves) and the attention matmul reads shifted windows of S directly
as the moving operand:

  O^T[din, ij] = sum_kl X[kl, din] * U^T[kl, ij]        (stage A, PE bf16)
  rhs[p, (i,j)] = S[p, 1764 - 252*c + 1008*n + 63*i + j]  for kl-chunk c
  (partition p corresponds to kl = 128*c + 127 - 32*(p%4) - p//4; X is
   pre-permuted on host to match; the permutation keeps the DMA's outer AP
   dim at count 32 so strip descriptors spread over all 16 DMA engines)

  V = O^T * (1/Z[ij])                                    (DVE, during PSUM copy)
  out[ij, dout] = sum_{h,din} V[(h,din), ij] * W^T[(h,din), dout] + b  (stage B)

Chunks whose Gaussian mass is below 1e-5 of the per-head peak are skipped.
Per core: 2 batches x 9 heads. ij split in two halves of 512 to bound PSUM.
"""
import numpy as np
import ml_dtypes

import concourse.bacc as bacc
import concourse.bass as bass
import concourse.mybir as mybir
from concourse.tile import TileContext
from concourse.bass_utils import run_bass_kernel_spmd

B, W_IMG, H_IMG, D = 16, 32, 32, 256
NH = 9
S = W_IMG * H_IMG          # 1024 positions
NCORES = 8
BLOC = B // NCORES         # batches per core
TBL = 63 * 63              # 3969
STRIP = 3749               # single wide strip per head (covers both halves)
F32 = mybir.dt.float32
BF16 = mybir.dt.bfloat16
BFNP = ml_dtypes.bfloat16

LAST_RESULT = None         # BassKernelResults of the most recent run (for test.py)


def _host_prep(attention_centers, attention_spreads, value_w):
    """u -> stabilized exp tables (63x63 per head, flipped layout), 1/Z,
    packed W^T, and the per-(head, ij-half) list of active kl-chunks."""
    ac = np.asarray(attention_centers, dtype=np.float32)
    sp = np.asarray(attention_spreads, dtype=np.float32)
    inv_cov = np.einsum("hij,hkj->hik", sp, sp).astype(np.float32)
    a, bb, c = inv_cov[:, 0, 0], inv_cov[:, 0, 1], inv_cov[:, 1, 1]
    mu1, mu2 = ac[:, 0], ac[:, 1]
    u1 = a * mu1 + bb * mu2
    u2 = c * mu2 + bb * mu1
    u3 = -0.5 * a
    u4 = -0.5 * c
    u5 = -bb

    # tab[h, 63*X + B] = exp(score(dx=31-X, dy=31-B) - max_h)
    dx = (31 - np.arange(63, dtype=np.float32))[:, None]
    dy = (31 - np.arange(63, dtype=np.float32))[None, :]
    sc = (u1[:, None, None] * dx + u2[:, None, None] * dy
          + u3[:, None, None] * dx * dx + u4[:, None, None] * dy * dy
          + u5[:, None, None] * dx * dy).astype(np.float32)
    sc -= sc.max(axis=(1, 2), keepdims=True)
    tab2d = np.exp(sc.astype(np.float64)).astype(np.float32)   # [9, 63, 63]

    # Z[h, iq, jq] = sum over the 32x32 window tab2d[h, iq:iq+32, jq:jq+32]
    cs = tab2d.astype(np.float64).cumsum(axis=1).cumsum(axis=2)
    cs = np.pad(cs, ((0, 0), (1, 0), (1, 0)))
    i0 = np.arange(32)
    zi, zj = np.meshgrid(i0, i0, indexing="ij")
    z = (cs[:, zi + 32, zj + 32] - cs[:, zi, zj + 32]
         - cs[:, zi + 32, zj] + cs[:, zi, zj])
    rz = (1.0 / z).reshape(NH, S).astype(BFNP)

    # active kl-chunks per (h, ij-half): chunk c covers k in [4c, 4c+4);
    # the (i, k) pairs it feeds for half n use table rows dx = k - i with
    # k in the chunk, i in [16n, 16n+16).  Skip chunks whose max table
    # entry over those rows is < 1e-5 (peak is 1.0 per head).
    rowmax = tab2d.max(axis=2)                     # [9, 63] over dy
    active = [[[] for _ in range(2)] for _ in range(NH)]
    for h in range(NH):
        for n in range(2):
            for cc in range(8):
                mx = 0.0
                for k in range(4 * cc, 4 * cc + 4):
                    for i in range(16 * n, 16 * n + 16):
                        mx = max(mx, rowmax[h, 31 - (k - i)])
                if mx >= 1e-5:
                    active[h][n].append(cc)
            assert active[h][n], (h, n)

    vw = np.asarray(value_w, dtype=np.float32)                 # [256, 2304]
    wt = np.ascontiguousarray(
        vw.reshape(D, NH, 2, 128).transpose(3, 1, 2, 0).reshape(128, NH * 2, D)
    ).astype(BFNP)
    return tab2d.reshape(NH, TBL).astype(BFNP).copy(), rz, wt, active


def _build_program(active):
    nc = bacc.Bacc("TRN2", target_bir_lowering=False, debug=False, enable_partition_id=False)
    x_d = nc.declare_dram_parameter("x", [128, BLOC, 8, D], BF16, isOutput=False)
    wt_d = nc.declare_dram_parameter("wt", [128, NH * 2, D], BF16, isOutput=False)
    tab_d = nc.declare_dram_parameter("tab", [NH, TBL], BF16, isOutput=False)
    rz_d = nc.declare_dram_parameter("rz", [NH, S], BF16, isOutput=False)
    vb_d = nc.declare_dram_parameter("vb", [D], F32, isOutput=False)
    y_d = nc.declare_dram_parameter("y", [BLOC, S, D], F32, isOutput=True)

    with TileContext(nc) as tc:
        with tc.tile_pool(name="singles", bufs=1) as singles, \
             tc.tile_pool(name="vs", bufs=38) as vpool, \
             tc.tile_pool(name="outs", bufs=3) as opool, \
             tc.tile_pool(name="pa", bufs=4, space="PSUM") as pa, \
             tc.tile_pool(name="pb", bufs=3, space="PSUM") as pb:

            # Single sync-queue, issue order = consumption order.  strip0 is
            # split so the n=0-relevant columns land first; the per-(0,h) rz
            # tiles are tiny and ride between strips.  All big transfers
            # keep an outer AP dim >= 16 so descriptors spread over all 16
            # DMA engines.
            x_sb = singles.tile([128, BLOC, 8, D], BF16)
            nc.sync.dma_start(out=x_sb[:, 0], in_=x_d[:, 0])

            strips = []
            rzt = {}

            def load_strip(h):
                s_t = singles.tile([128, STRIP], BF16, tag=f"s{h}", name=f"s{h}")
                strips.append(s_t)
                nc.sync.dma_start(
                    out=s_t,
                    in_=bass.AP(tensor=tab_d, offset=h * TBL,
                                ap=[[1, 32], [63, 4], [1, STRIP]]))

            def load_rz(h):
                t = singles.tile([128, S], BF16, tag=f"rz{h}", name=f"rz{h}")
                nc.sync.dma_start(
                    out=t, in_=bass.AP(tensor=rz_d, offset=h * S,
                                       ap=[[0, 128], [1, S]]))
                rzt[h] = t

            load_strip(0)
            load_rz(0)
            nc.sync.dma_start(out=x_sb[:, 1], in_=x_d[:, 1])
            for h in range(1, NH):
                load_strip(h)
                load_rz(h)
            wt_sb = singles.tile([128, NH * 2, D], BF16)
            nc.sync.dma_start(out=wt_sb, in_=wt_d[:, :, :])
            vb_sb = singles.tile([128, D], F32)
            nc.sync.dma_start(
                out=vb_sb, in_=bass.AP(tensor=vb_d, offset=0, ap=[[0, 128], [1, D]]))

            for n in range(2):                       # ij half
                vt = {}
                for h in range(NH):
                    s_t = strips[h]
                    rz_t = rzt[h][:, 512 * n:512 * n + 512]
                    cs = active[h][n]
                    for b in range(BLOC):
                        for m in range(2):           # din chunk
                            ps = pa.tile([128, 512], F32, tag="pa", name="ps")
                            for ci, c in enumerate(cs):
                                rhs = bass.AP(
                                    tensor=s_t.tensor,
                                    offset=s_t.offset + (1764 - 252 * c + 1008 * n),
                                    ap=[s_t.ap[0], [63, 16], [1, 32]])
                                nc.tensor.matmul(
                                    ps,
                                    lhsT=x_sb[:, b, c, m * 128:(m + 1) * 128],
                                    rhs=rhs, start=(ci == 0),
                                    stop=(ci == len(cs) - 1))
                            v = vpool.tile([128, 512], BF16, tag="v")
                            nc.vector.tensor_mul(v, ps, rz_t)
                            vt[(b, 2 * h + m)] = v
                # stage B: output linear, accumulate over (h, din)
                for b in range(BLOC):
                    for r in range(4):               # ij chunk within half
                        po = pb.tile([128, D], F32, tag="pb", name="po")
                        for q in range(NH * 2):
                            nc.tensor.matmul(
                                po, lhsT=vt[(b, q)][:, r * 128:(r + 1) * 128],
                                rhs=wt_sb[:, q, :],
                                start=(q == 0), stop=(q == NH * 2 - 1))
                        ot = opool.tile([128, D], F32, tag="ot")
                        nc.vector.tensor_add(ot, po, vb_sb)
                        row = 512 * n + 128 * r
                        nc.sync.dma_start(out=y_d[b, row:row + 128, :], in_=ot)
    nc.compile()
    return nc


def kernel(hidden_states, attention_mask, attention_centers, attention_spreads,
           value_w, value_b, **_ignored):
    global LAST_RESULT
    hs = np.asarray(hidden_states, dtype=np.float32)
    tab, rz, wt, active = _host_prep(attention_centers, attention_spreads, value_w)
    vb = np.ascontiguousarray(np.asarray(value_b, dtype=np.float32))

    # per-core x: partition p of chunk c holds kl = 128c + 127 - 32*(p%4) - p//4
    p = np.arange(128)
    perm = 127 - 32 * (p % 4) - p // 4
    xr = hs.reshape(B, 8, 128, D)[:, :, perm, :]
    in_maps = []
    for cid in range(NCORES):
        xc = np.ascontiguousarray(
            xr[cid * BLOC:(cid + 1) * BLOC].transpose(2, 0, 1, 3)).astype(BFNP)
        in_maps.append({"x": xc, "wt": wt, "tab": tab, "rz": rz, "vb": vb})

    nc = _build_program(active)
    LAST_RESULT = run_bass_kernel_spmd(nc, in_maps, core_ids=list(range(NCORES)))

    out = np.concatenate([r["y"] for r in LAST_RESULT.results], axis=0)
    return out.reshape(B, W_IMG, H_IMG, D)
